# revision 72
# baseline (speedup 1.0000x reference)
"""Trainium2 Bass kernel for nn_AttentionLayer (B=8, S=1024, D=1024, H=16, HD=64).

Strategy: pure data parallelism - one batch element per NeuronCore (8 cores).
Weights replicated (pre-transposed on host so the contraction dim lands on SBUF
partitions); x sharded on batch and pre-transposed per shard.

Optimizations vs the 444us baseline (trace-driven; final HW exec ~278us,
rel err 1.08e-2 against the 2e-2 gate):
  * Q/K projections in fp8e4 DoubleRow (K=256 per instruction, half the PE
    columns; measured 379ns per 512-col on HW = exactly 2x bf16). x and
    Wq/Wk host-quantized with power-of-2 scales (SX=32, SW=4096); dequant
    folds into the psum->SBUF bias-add (tensor_scalar mult+add). V/O/AV
    must stay bf16: fp8 there measured 2.7e-2..5.7e-2 offline.
  * Q/K projection for pair t+2 interleaved into the attention loop, split
    around the denominator broadcast so the PE always has ready work while
    ACT drains psums / DVE runs the reciprocal. The attention phase alone is
    ACT(exp)-bound; the proj fill keeps the PE the bottleneck.
  * Pair 0's scores+exp+mask interleaved into the V-projection m-loop: ACT
    warms up during phase V and the attention pipeline prologue hides
    entirely (-35us).
  * Causal mask identity-matmuls dropped from the PE (-24us): exp writes
    [128c, S) per key-tile (odd tiles' dead first 128 cols never computed),
    the 128-col diagonal triangle gets a 0/1 multiply on DVE, and the static
    zero region [0, 128c) of each persistent ex tile is memset once.
  * AV matmuls trimmed to the causally valid column range per key tile
    (ex is zero below 128c), -8us.
  * Denominator path with no DRAM bounce: AV's ones-column yields the sum
    row in psum; ACT drains it to st; a K=1 ones-matmul at tile_position
    (64, par*64) broadcasts it to the head's 64 psum partitions;
    DVE reciprocal_approx_fast (5x faster than reciprocal) produces rec;
    GPSIMD does the normalize multiply. The baseline's DRAM round-trip
    head-of-line-blocked the in-order DVE queue (7-17us/pair stalls).
  * Explicit WAR deps guard reciprocal_approx_fast's psum read (custom-DVE
    ops are invisible to the tile framework's dependency tracking - this
    raced and corrupted denominators on HW while passing CoreSim).
  * Phase O: psc pool closed first so the output psums get all 8 banks;
    drains alternate ACT/DVE to halve the tail.

Hard-won HW facts: PE column rate is capped at ~0.74ns/col (50% util
throttle; p-state never reaches 2.4GHz, so column count - not FLOPs - is
the real currency); fp8 DoublePixel mode is a no-op; GPSIMD cannot touch
PSUM; engines cannot shift partitions (only DMA and PE matmuls can);
SBUF APs cannot have partition stride 0; gpsimd.partition_broadcast writes
wrong partitions for base!=0 outputs; f32r matmuls hang the chip.
"""

import os
import sys
import types

import numpy as np

B, S, D, H, HD = 8, 1024, 1024, 16, 64
NT = D // 128          # 8 partition tiles
NP = NT // 2           # 4 DoubleRow pair tiles
PAD_ID = 1.0
SCALE = 1.0 / 8.0      # 1/sqrt(HD)
SX = 32.0              # x fp8 scale (absmax ~4.9 -> 155 < 240)
SW = 4096.0            # Wq/Wk fp8 scale (absmax 1/32 -> 128 < 240)
INV_QK = 1.0 / (SX * SW)

_CACHE = {}
LAST_RESULT = None
LAST_EXEC_NS = None


def _install_trace_hook():
    """Provide antenv.axon_hooks (missing in this image) so trace=True works."""
    try:
        import antenv
        if "antenv.axon_hooks" in sys.modules:
            return True
        m = types.ModuleType("antenv.axon_hooks")
        _hook = [None]
        m.set_axon_ntff_profile_hook = lambda h: _hook.__setitem__(0, h)
        m.get_axon_ntff_profile_hook = lambda: _hook[0]
        sys.modules["antenv.axon_hooks"] = m
        antenv.axon_hooks = m
        from trn_agent_boot.trn_boot import _ntff_profile_via_ctypes
        hook = _ntff_profile_via_ctypes("/opt/axon/libaxon_pjrt.so")
        if hook is None:
            return False
        m.set_axon_ntff_profile_hook(hook)
        return True
    except Exception:
        return False


def _build_graph():
    import concourse.bass as bass
    import concourse.mybir as mybir
    import concourse.tile as tile
    from concourse import bacc

    F32 = mybir.dt.float32
    BF16 = mybir.dt.bfloat16
    F8 = mybir.dt.float8e4
    AluOp = mybir.AluOpType
    Act = mybir.ActivationFunctionType
    DR = mybir.MatmulPerfMode.DoubleRow

    nc = bacc.Bacc(target_bir_lowering=False)

    def dep(later, earlier, reason):
        bass._add_dep_helper(later.ins, earlier.ins, reason=reason)

    # fp8 operands, DoubleRow pair-interleaved on host: row block j holds
    # d-rows [256j, 256j+128) as subtile 0 and [256j+128, 256j+256) as 1.
    x8_e = nc.declare_dram_parameter("x8", [NP * 128, 2 * S], F8, isOutput=False)
    wq8_e = nc.declare_dram_parameter("wq8", [NP * 128, 2 * D], F8, isOutput=False)
    wk8_e = nc.declare_dram_parameter("wk8", [NP * 128, 2 * D], F8, isOutput=False)
    xT = nc.declare_dram_parameter("xT", [D, S], BF16, isOutput=False)
    WvT = nc.declare_dram_parameter("WvT", [D, D], BF16, isOutput=False)
    WoT = nc.declare_dram_parameter("WoT", [D, D], BF16, isOutput=False)
    bv = nc.declare_dram_parameter("bv", [D], BF16, isOutput=False)
    bo = nc.declare_dram_parameter("bo", [D], BF16, isOutput=False)
    ones_p = nc.declare_dram_parameter("ones", [S], BF16, isOutput=False)
    # smalls: [128, 24] f32 = ids_r | bq_r | bk_r (each [128, 8], host-packed)
    smalls = nc.declare_dram_parameter("smalls", [128, 3 * NT], F32, isOutput=False)
    # 0/1 causal triangle for the diagonal 128-col window: 1 where col >= row
    tri_p = nc.declare_dram_parameter("tri01", [128, 128], BF16, isOutput=False)
    out_e = nc.declare_dram_parameter("out", [S, D], F32, isOutput=True)
    DBG = os.environ.get("KERNEL_DEBUG", "0") == "1"
    if DBG:
        dbg_vx = nc.declare_dram_parameter("dbg_vx", [128, H * 2 * HD], BF16,
                                           isOutput=True)
        dbg_ex = nc.declare_dram_parameter("dbg_ex", [128, S], BF16,
                                           isOutput=True)
        dbg_st = nc.declare_dram_parameter("dbg_st", [128, S], BF16,
                                           isOutput=True)
        dbg_rec = nc.declare_dram_parameter("dbg_rec", [128, S], F32,
                                            isOutput=True)
        dbg_ao = nc.declare_dram_parameter("dbg_ao", [128, S], BF16,
                                           isOutput=True)
        dbg_av = nc.declare_dram_parameter("dbg_av", [128, S], F32,
                                           isOutput=True)

    with tile.TileContext(nc) as tc:
        with tc.tile_pool(name="const", bufs=1) as cp, \
             tc.tile_pool(name="persist", bufs=1) as qp, \
             tc.tile_pool(name="wo", bufs=8) as wop:

            # ---- constants ----
            sm = cp.tile([128, 3 * NT], F32, tag="sm", name="sm")
            nc.sync.dma_start(out=sm[:], in_=smalls[:])
            pad01 = cp.tile([128, NT], F32, tag="pad01", name="pad01")
            nc.vector.tensor_scalar(out=pad01[:], in0=sm[:, 0:NT],
                                    scalar1=PAD_ID, scalar2=None,
                                    op0=AluOp.not_equal)
            bq_col = sm[:, NT:2 * NT]
            bk_col = sm[:, 2 * NT:3 * NT]
            # bias images broadcast across partitions (DRAM-source
            # stride-0 partition reads are legal, unlike SBUF ones); the
            # ones-matmul bias accumulation becomes a free add in the drain
            bv_im = cp.tile([128, D], BF16, tag="bvim", name="bv_im")
            nc.sync.dma_start(out=bv_im[:],
                              in_=bv[None, :].broadcast_to([128, D]))
            bo_im = cp.tile([128, D], BF16, tag="boim", name="bo_im")
            nc.sync.dma_start(out=bo_im[:],
                              in_=bo[None, :].broadcast_to([128, D]))
            tri = cp.tile([128, 128], BF16, tag="tri", name="tri")
            nc.sync.dma_start(out=tri[:], in_=tri_p[:])
            # all-ones block; row 64 feeds the K=1 denominator broadcast
            # matmul (lhsT must sit at partition 64 = st's denominator row)
            ones_c = cp.tile([128, 64], BF16, tag="onesc", name="ones_c")
            nc.vector.memset(ones_c[:], 1.0)

            # fp8 Q/K proj operands (small: 0.25MB + 2x1MB)
            x8_sb = [qp.tile([128, 2, S], F8, tag=f"x8{j}", name=f"x8{j}")
                     for j in range(NP)]
            wq8_sb = [qp.tile([128, 2, D], F8, tag=f"wq8{j}", name=f"wq8{j}")
                      for j in range(NP)]
            wk8_sb = [qp.tile([128, 2, D], F8, tag=f"wk8{j}", name=f"wk8{j}")
                      for j in range(NP)]
            for j in range(NP):
                nc.sync.dma_start(
                    out=x8_sb[j][:], in_=x8_e[j * 128:(j + 1) * 128, :])
            for j in range(NP):
                nc.sync.dma_start(
                    out=wq8_sb[j][:], in_=wq8_e[j * 128:(j + 1) * 128, :])
                nc.sync.dma_start(
                    out=wk8_sb[j][:], in_=wk8_e[j * 128:(j + 1) * 128, :])

            # ---- persistent per-core tensors ----
            Vx = [qp.tile([128, H * (HD + 1)], BF16, tag=f"vx{t}", name=f"vx{t}")
                  for t in range(NT)]
            aoT = [qp.tile([128, S], BF16, tag=f"ao{t}", name=f"ao{t}")
                   for t in range(NT)]
            # persistent exp tiles keyed by (key-tile c, head-in-pair par);
            # [0, 128c) is a static zero region, memset once here.
            exs = {}
            for c in range(NT):
                for par in range(2):
                    ex = qp.tile([128, S], BF16, tag=f"ex{c}_{par}",
                                 name=f"ex{c}_{par}")
                    exs[(c, par)] = ex
                    if c > 0:
                        nc.gpsimd.memset(ex[:, 0:128 * c], 0.0)

            # Pools shared by Q/K proj (fp8 DR), attention and epilogue.
            with tc.tile_pool(name="qk", bufs=3) as qkp, \
                 tc.tile_pool(name="stp", bufs=2) as stp, \
                 tc.tile_pool(name="rec", bufs=2) as recp:
                psc_cm = tc.tile_pool(name="pssc", bufs=2, space="PSUM")
                psc = psc_cm.__enter__()

                # WAR guard: the custom-DVE reciprocal's PSUM read of the bc
                # tile is NOT tracked by the tile framework (observed racing
                # on HW). The sc pool has bufs=2, so the second sc tile
                # allocated after bc reuses its buffer; its first matmul must
                # explicitly wait on the reciprocal.
                sc_state = {"count": 0, "pending": {}}

                def alloc_sc():
                    sc_state["count"] += 1
                    return (psc.tile([128, 1024], F32, tag="sc", name="sc"),
                            sc_state["pending"].pop(sc_state["count"], None))

                qt_tiles, kt_tiles = {}, {}

                def qk_proj_one(m, which):
                    """fp8 DoubleRow projection of Q or K output tile m."""
                    w8, dst_map, bias_col, tg = (
                        (wq8_sb, qt_tiles, bq_col, "qt") if which == 0
                        else (wk8_sb, kt_tiles, bk_col, "kt"))
                    ps, war = alloc_sc()
                    for n in range(2):
                        for j in range(NP):
                            mm = nc.tensor.matmul(
                                ps[:, n * 512:(n + 1) * 512],
                                w8[j][:, :, m * 128:(m + 1) * 128],
                                x8_sb[j][:, :, n * 512:(n + 1) * 512],
                                start=(j == 0), stop=(j == NP - 1),
                                perf_mode=DR)
                            if war is not None:
                                dep(mm, war, reason="WAR: bc recip read")
                                war = None
                    dst = qkp.tile([128, S], BF16, tag=tg, name=tg)
                    dst_map[m] = dst
                    nc.vector.tensor_scalar(
                        out=dst[:], in0=ps[:],
                        scalar1=INV_QK, scalar2=bias_col[:, m:m + 1],
                        op0=AluOp.mult, op1=AluOp.add)

                def qk_proj(m):
                    qk_proj_one(m, 0)
                    qk_proj_one(m, 1)

                # Q/K proj of the first two pairs needs only the small fp8
                # inputs - runs while the 4MB of bf16 x/Wv still streams in.
                qk_proj(0)
                qk_proj(1)

                def scores_block(t, c, par):
                    """Scores + exp + causal mask for (pair t, key tile c)."""
                    Qt, Kt = qt_tiles[t], kt_tiles[t]
                    qs = 256 * (c // 2)      # psum tile covers [qs, S)
                    q0 = 128 * c             # first valid query col
                    base = par * 64
                    sc, war = alloc_sc()
                    n0 = q0
                    while n0 < S:
                        n1 = min(qs + 512 * ((n0 - qs) // 512 + 1), S)
                        mm = nc.tensor.matmul(
                            sc[:, n0 - qs:n1 - qs],
                            Kt[base:base + 64, c * 128:(c + 1) * 128],
                            Qt[base:base + 64, n0:n1],
                            start=True, stop=True)
                        if war is not None:
                            dep(mm, war, reason="WAR: bc recip read")
                            war = None
                        n0 = n1
                    ex = exs[(c, par)]
                    nc.scalar.activation(out=ex[:, q0:S],
                                         in_=sc[:, q0 - qs:S - qs],
                                         func=Act.Exp, scale=SCALE)
                    # causal triangle on the diagonal 128-col window
                    nc.vector.tensor_mul(ex[:, q0:q0 + 128],
                                         ex[:, q0:q0 + 128], tri[:])

                # ============ Phase V: V projection (bf16) ============
                with tc.tile_pool(name="xv", bufs=1) as xp, \
                     tc.tile_pool(name="wst", bufs=8) as wp, \
                     tc.tile_pool(name="psv", bufs=4, space="PSUM") as pvp:

                    # interleave x/Wv tile loads so V-proj's c-chain can
                    # start as soon as the matching pair of tiles lands
                    xT_sb = [xp.tile([128, S], BF16, tag=f"x{c}", name=f"x{c}")
                             for c in range(NT)]
                    wv_sb = [wp.tile([128, D], BF16, tag="wv", name="wv_t")
                             for c in range(NT)]
                    for c in range(NT):
                        nc.sync.dma_start(out=xT_sb[c][:],
                                          in_=xT[c * 128:(c + 1) * 128, :])
                        nc.sync.dma_start(out=wv_sb[c][:],
                                          in_=WvT[c * 128:(c + 1) * 128, :])
                    for m in range(NT):
                        vdst = Vx[m][:].rearrange("p (h e) -> p h e", e=HD + 1)
                        nc.vector.memset(vdst[:, :, HD:HD + 1], 1.0)
                        for n in range(2):
                            ps = pvp.tile([128, 512], F32, tag="pv", name="pv")
                            for c in range(NT):
                                nc.tensor.matmul(
                                    ps[:],
                                    xT_sb[c][:, m * 128:(m + 1) * 128],
                                    wv_sb[c][:, n * 512:(n + 1) * 512],
                                    start=(c == 0), stop=(c == NT - 1))
                            nc.vector.tensor_add(
                                vdst[:, n * 8:(n + 1) * 8, 0:HD],
                                ps[:].rearrange("p (h e) -> p h e", e=HD),
                                bv_im[:, n * 512:(n + 1) * 512].rearrange(
                                    "p (h e) -> p h e", e=HD))
                        # pad mask: zero whole key rows where ids == PAD,
                        # incl. the ones column -> denominator excludes them
                        nc.vector.tensor_scalar(
                            out=Vx[m][:], in0=Vx[m][:],
                            scalar1=pad01[:, m:m + 1], scalar2=None,
                            op0=AluOp.mult)
                        # interleave pair 0's scores/exp so ACT warms up
                        # during the V projection (its AV runs in the loop)
                        scores_block(0, m, 0)
                        scores_block(0, m, 1)

                # ==== Phase A: attention, Q/K proj interleaved per pair ====
                # prefetch Wo during attention
                wo_sb = []
                for c in range(NT):
                    w_t = wop.tile([128, D], BF16, tag="wo", name="wo_t")
                    nc.sync.dma_start(out=w_t[:],
                                      in_=WoT[c * 128:(c + 1) * 128, :])
                    wo_sb.append(w_t)

                attn_stack = tc.tile_pool(name="psav", bufs=1, space="PSUM")
                pav = attn_stack.__enter__()
                for t in range(NT):        # head pair (2t, 2t+1)
                    Qt, Kt = qt_tiles[t], kt_tiles[t]
                    av_ps = {(par, g): pav.tile([HD + 1, 512], F32,
                                                tag=f"av{par}{g}",
                                                name=f"av{par}{g}")
                             for par in range(2) for g in range(2)}
                    for c in range(NT):
                        q0 = 128 * c             # first valid query col
                        for par in range(2):
                            if t > 0:
                                scores_block(t, c, par)
                            ex = exs[(c, par)]
                            for g in range(2):
                                if c <= 4 * g + 3:
                                    h = 2 * t + par
                                    # trim the chunk to the causally valid
                                    # columns (ex is zero below q0)
                                    o = max(0, q0 - 512 * g)
                                    nc.tensor.matmul(
                                        av_ps[(par, g)][:, o:512],
                                        Vx[c][:, h * (HD + 1):
                                               (h + 1) * (HD + 1)],
                                        ex[:, 512 * g + o:512 * (g + 1)],
                                        start=(c == 0),
                                        stop=(c == min(4 * g + 3, NT - 1)))

                    # epilogue: drain AV psums on ACT (reads PSUM; keeps DVE
                    # free). Denominator row (st partition 64) is broadcast
                    # to 64 partitions per head via a K=1 ones-matmul into a
                    # scores-pool psum tile, reciprocal'd on DVE, and the
                    # normalize runs on the idle GPSIMD - no DMA in the chain.
                    # epilogue: drain AV psums on ACT, broadcast the
                    # denominator row via a K=1 ones-matmul into an sc-pool
                    # psum tile, fast-reciprocal on DVE, normalize on GPSIMD
                    sts = {}
                    for par in range(2):
                        st = stp.tile([HD + 1, S], BF16, tag=f"st{par}",
                                      name=f"st{par}")
                        sts[par] = st
                        for g in range(2):
                            nc.scalar.copy(
                                out=st[:, 512 * g:512 * (g + 1)],
                                in_=av_ps[(par, g)][:])
                        nc.sync.dma_start(
                            out=aoT[t][par * 64:par * 64 + HD, :],
                            in_=st[0:HD, :])

                    if t + 2 < NT:
                        qk_proj_one(t + 2, 0)   # covers the drain latency

                    bc, war = alloc_sc()
                    for par in range(2):
                        for n in range(2):
                            mm = nc.tensor.matmul(
                                bc[par * 64:par * 64 + 64,
                                   n * 512:(n + 1) * 512],
                                ones_c[HD:HD + 1, 0:64],
                                sts[par][HD:HD + 1, n * 512:(n + 1) * 512],
                                start=True, stop=True,
                                tile_position=(64, par * 64))
                            if war is not None:
                                dep(mm, war, reason="WAR: bc recip read")
                                war = None

                    rec = recp.tile([128, S], F32, tag="rec", name="rec")
                    rcp = nc.vector.reciprocal_approx_fast(out=rec[:],
                                                           in_=bc[:])

                    if t + 2 < NT:
                        qk_proj_one(t + 2, 1)   # covers the reciprocal latency
                    sc_state["pending"][sc_state["count"] + 2] = rcp
                    if DBG and t == 0:
                        nc.sync.dma_start(out=dbg_vx[:, 0:H * (HD + 1)],
                                          in_=Vx[0][:])
                        nc.sync.dma_start(out=dbg_ex[:], in_=exs[(0, 0)][:])
                        nc.sync.dma_start(out=dbg_st[0:HD + 1, :],
                                          in_=sts[0][:])
                        nc.sync.dma_start(out=dbg_rec[:], in_=rec[:])
                    nc.gpsimd.tensor_mul(aoT[t][:], aoT[t][:], rec[:])
                    if DBG and t == 0:
                        nc.sync.dma_start(out=dbg_ao[:], in_=aoT[0][:])
                attn_stack.__exit__(None, None, None)
                psc_cm.__exit__(None, None, None)

            # ============ Phase O: output projection ============
            # (first chunks' matmuls guard the last bc reciprocal's untracked
            # psum read before its banks are recycled into the pf pool)
            with tc.tile_pool(name="ost", bufs=4) as osp, \
                 tc.tile_pool(name="psf", bufs=8, space="PSUM") as pf:
                for m in range(NT):
                    for n in range(2):
                        ps = pf.tile([128, 512], F32, tag="pf", name="psf")
                        for c in range(NT):
                            mm = nc.tensor.matmul(
                                ps[:],
                                aoT[c][:, m * 128:(m + 1) * 128],
                                wo_sb[c][:, n * 512:(n + 1) * 512],
                                start=(c == 0), stop=(c == NT - 1))
                            if m <= 1 and c == 0 and rcp is not None:
                                dep(mm, rcp, reason="WAR: last bc recip")
                        ot = osp.tile([128, 512], F32, tag="ot", name="ot")
                        nc.vector.tensor_add(
                            ot[:], ps[:], bo_im[:, n * 512:(n + 1) * 512])
                        nc.sync.dma_start(
                            out=out_e[m * 128:(m + 1) * 128,
                                      n * 512:(n + 1) * 512],
                            in_=ot[:])
    nc.finalize()
    return nc


def _host_consts():
    import ml_dtypes
    bf = ml_dtypes.bfloat16
    jj = np.arange(128)[None, :]
    pp = np.arange(128)[:, None]
    tri01 = (jj >= pp).astype(np.float32).astype(bf)   # 1 where col >= row
    return tri01


def _to_f8(a, scale):
    import ml_dtypes
    return np.asarray(
        np.clip(np.asarray(a, np.float32) * scale, -240.0, 240.0),
        dtype=ml_dtypes.float8_e4m3)


def _pair_interleave(a):
    """[D, N] -> [NP*128, 2*N]: row block j = (d rows 256j..+128 | ..+256)."""
    d, n = a.shape
    return np.ascontiguousarray(
        a.reshape(NP, 2, 128, n).transpose(0, 2, 1, 3).reshape(NP * 128, 2 * n))


def build_in_maps(x, input_ids, Wq, bq, Wk, bk, Wv, bv, Wo, bo):
    import ml_dtypes
    bf = ml_dtypes.bfloat16
    x = np.asarray(x, dtype=np.float32)
    input_ids = np.asarray(input_ids)
    tri01 = _host_consts()
    bq_r = np.ascontiguousarray(np.asarray(bq, np.float32).reshape(NT, 128).T)
    bk_r = np.ascontiguousarray(np.asarray(bk, np.float32).reshape(NT, 128).T)
    wq8 = _pair_interleave(_to_f8(np.asarray(Wq, np.float32).T, SW))
    wk8 = _pair_interleave(_to_f8(np.asarray(Wk, np.float32).T, SW))
    shared = {
        "wq8": wq8, "wk8": wk8,
        "WvT": np.ascontiguousarray(np.asarray(Wv, np.float32).T).astype(bf),
        "WoT": np.ascontiguousarray(np.asarray(Wo, np.float32).T).astype(bf),
        "bv": np.asarray(bv, np.float32).astype(bf),
        "bo": np.asarray(bo, np.float32).astype(bf),
        "ones": np.ones([S], bf),
        "tri01": tri01,
    }
    in_maps = []
    for b in range(B):
        ids_r = input_ids[b].astype(np.float32).reshape(NT, 128).T
        m = dict(shared)
        xb_T = np.ascontiguousarray(x[b].T)
        m["xT"] = xb_T.astype(bf)
        m["x8"] = _pair_interleave(_to_f8(xb_T, SX))
        m["smalls"] = np.ascontiguousarray(
            np.concatenate([ids_r, bq_r, bk_r], axis=1)).astype(np.float32)
        in_maps.append(m)
    return in_maps


def kernel(x, input_ids, Wq, bq, Wk, bk, Wv, bv, Wo, bo):
    global LAST_RESULT, LAST_EXEC_NS
    from concourse.bass_utils import run_bass_kernel_spmd

    if "nc" not in _CACHE:
        _CACHE["nc"] = _build_graph()
    nc = _CACHE["nc"]
    in_maps = build_in_maps(x, input_ids, Wq, bq, Wk, bk, Wv, bv, Wo, bo)

    trace = os.environ.get("KERNEL_TRACE", "0") == "1" and _install_trace_hook()
    res = run_bass_kernel_spmd(nc, in_maps, core_ids=list(range(B)), trace=trace)
    LAST_RESULT = res
    LAST_EXEC_NS = res.exec_time_ns
    return np.stack([res.results[b]["out"] for b in range(B)]).astype(np.float32)


# revision 73
# speedup vs baseline: 1.1949x; 1.1949x over previous
"""Trainium2 Bass kernel for nn_AttentionLayer (B=8, S=1024, D=1024, H=16, HD=64).

Strategy: pure data parallelism - one batch element per NeuronCore (8 cores).
Weights replicated (pre-transposed on host so the contraction dim lands on SBUF
partitions); x sharded on batch and pre-transposed per shard.

v2 changes vs the 444us baseline (trace-driven):
  * Q/K projections in fp8e4 DoubleRow (K=256 per instruction, half the PE
    columns). x and Wq/Wk host-quantized with power-of-2 scales; dequant is
    folded into the psum->SBUF bias-add (tensor_scalar mult+add). End-to-end
    rel err ~1.1e-2 (budget 2e-2); V/O/AV stay bf16 - fp8 there measured
    2.7e-2..5.7e-2 offline.
  * Q/K projection for pair t+1 interleaved into the attention loop: the
    attention phase alone is ACT(exp)-bound (12.8us exp vs 11.3us PE per
    pair); adding 6us/pair of proj matmuls keeps the PE the bottleneck and
    the clock ramped.
  * Causal mask identity-matmuls dropped from the PE (-24us): exp writes
    [128c, S) per key-tile (odd tiles' fully-dead first 128 cols never
    computed), the 128-col diagonal triangle gets a 0/1 multiply on DVE, and
    the static zero region [0, 128c) of each persistent ex tile is memset
    once at startup.
  * AV-psum drains, ex gap zeroing and the softmax normalize moved from DVE
    to the idle GPSIMD engine: the per-pair denominator DMA round-trip no
    longer head-of-line-blocks the in-order DVE queue, which was stalling the
    next pair's matmuls on PSUM WAR (7-17us/pair in the baseline trace).
  * PSUM: scores/proj share one [128,1024]x2 pool (4 banks) + 4 AV banks.

Matmul dtype: bf16 (f32r hangs TRN2 hardware - observed empirically); fp8e4
DoubleRow only for Q/K proj.
"""

import os
import sys
import types

import numpy as np

B, S, D, H, HD = 8, 1024, 1024, 16, 64
NT = D // 128          # 8 partition tiles
NP = NT // 2           # 4 DoubleRow pair tiles
PAD_ID = 1.0
SCALE = 1.0 / 8.0      # 1/sqrt(HD)
SX = 32.0              # x fp8 scale (absmax ~4.9 -> 155 < 240)
SW = 4096.0            # Wq/Wk fp8 scale (absmax 1/32 -> 128 < 240)
INV_QK = 1.0 / (SX * SW)

_CACHE = {}
LAST_RESULT = None
LAST_EXEC_NS = None


def _install_trace_hook():
    """Provide antenv.axon_hooks (missing in this image) so trace=True works."""
    try:
        import antenv
        if "antenv.axon_hooks" in sys.modules:
            return True
        m = types.ModuleType("antenv.axon_hooks")
        _hook = [None]
        m.set_axon_ntff_profile_hook = lambda h: _hook.__setitem__(0, h)
        m.get_axon_ntff_profile_hook = lambda: _hook[0]
        sys.modules["antenv.axon_hooks"] = m
        antenv.axon_hooks = m
        from trn_agent_boot.trn_boot import _ntff_profile_via_ctypes
        hook = _ntff_profile_via_ctypes("/opt/axon/libaxon_pjrt.so")
        if hook is None:
            return False
        m.set_axon_ntff_profile_hook(hook)
        return True
    except Exception:
        return False


def _build_graph():
    import concourse.bass as bass
    import concourse.mybir as mybir
    import concourse.tile as tile
    from concourse import bacc

    F32 = mybir.dt.float32
    BF16 = mybir.dt.bfloat16
    F8 = mybir.dt.float8e4
    AluOp = mybir.AluOpType
    Act = mybir.ActivationFunctionType
    DR = mybir.MatmulPerfMode.DoubleRow

    nc = bacc.Bacc(target_bir_lowering=False)

    def dep(later, earlier, reason):
        bass._add_dep_helper(later.ins, earlier.ins, reason=reason)

    # fp8 operands, DoubleRow pair-interleaved on host: row block j holds
    # d-rows [256j, 256j+128) as subtile 0 and [256j+128, 256j+256) as 1.
    x8_e = nc.declare_dram_parameter("x8", [NP * 128, 2 * S], F8, isOutput=False)
    wq8_e = nc.declare_dram_parameter("wq8", [NP * 128, 2 * D], F8, isOutput=False)
    wk8_e = nc.declare_dram_parameter("wk8", [NP * 128, 2 * D], F8, isOutput=False)
    xT = nc.declare_dram_parameter("xT", [D, S], BF16, isOutput=False)
    WvT = nc.declare_dram_parameter("WvT", [D, D], BF16, isOutput=False)
    WoT = nc.declare_dram_parameter("WoT", [D, D], BF16, isOutput=False)
    bv = nc.declare_dram_parameter("bv", [D], BF16, isOutput=False)
    bo = nc.declare_dram_parameter("bo", [D], BF16, isOutput=False)
    ones_p = nc.declare_dram_parameter("ones", [S], BF16, isOutput=False)
    # smalls: [128, 24] f32 = ids_r | bq_r | bk_r (each [128, 8], host-packed)
    smalls = nc.declare_dram_parameter("smalls", [128, 3 * NT], F32, isOutput=False)
    # 0/1 causal triangle for the diagonal 128-col window: 1 where col >= row
    tri_p = nc.declare_dram_parameter("tri01", [128, 128], BF16, isOutput=False)
    out_e = nc.declare_dram_parameter("out", [S, D], F32, isOutput=True)
    DBG = os.environ.get("KERNEL_DEBUG", "0") == "1"
    if DBG:
        dbg_vx = nc.declare_dram_parameter("dbg_vx", [128, H * 2 * HD], BF16,
                                           isOutput=True)
        dbg_ex = nc.declare_dram_parameter("dbg_ex", [128, S], BF16,
                                           isOutput=True)
        dbg_st = nc.declare_dram_parameter("dbg_st", [128, S], BF16,
                                           isOutput=True)
        dbg_rec = nc.declare_dram_parameter("dbg_rec", [128, S], F32,
                                            isOutput=True)
        dbg_ao = nc.declare_dram_parameter("dbg_ao", [128, S], BF16,
                                           isOutput=True)
        dbg_av = nc.declare_dram_parameter("dbg_av", [128, S], F32,
                                           isOutput=True)

    with tile.TileContext(nc) as tc:
        with tc.tile_pool(name="const", bufs=1) as cp, \
             tc.tile_pool(name="persist", bufs=1) as qp, \
             tc.tile_pool(name="wo", bufs=8) as wop:

            # ---- constants ----
            sm = cp.tile([128, 3 * NT], F32, tag="sm", name="sm")
            nc.sync.dma_start(out=sm[:], in_=smalls[:])
            pad01 = cp.tile([128, NT], F32, tag="pad01", name="pad01")
            nc.vector.tensor_scalar(out=pad01[:], in0=sm[:, 0:NT],
                                    scalar1=PAD_ID, scalar2=None,
                                    op0=AluOp.not_equal)
            bq_col = sm[:, NT:2 * NT]
            bk_col = sm[:, 2 * NT:3 * NT]
            bv_row = cp.tile([1, D], BF16, tag="bvr", name="bv_row")
            nc.sync.dma_start(out=bv_row[:], in_=bv[None, :])
            bo_row = cp.tile([1, D], BF16, tag="bor", name="bo_row")
            nc.sync.dma_start(out=bo_row[:], in_=bo[None, :])
            ones_row = cp.tile([1, S], BF16, tag="ones", name="ones_row")
            nc.sync.dma_start(out=ones_row[:], in_=ones_p[None, :])
            tri = cp.tile([128, 128], BF16, tag="tri", name="tri")
            nc.sync.dma_start(out=tri[:], in_=tri_p[:])
            # all-ones block; row 64 feeds the K=1 denominator broadcast
            # matmul (lhsT must sit at partition 64 = st's denominator row)
            ones_c = cp.tile([128, 64], BF16, tag="onesc", name="ones_c")
            nc.vector.memset(ones_c[:], 1.0)

            # fp8 Q/K proj operands (small: 0.25MB + 2x1MB)
            x8_sb = [qp.tile([128, 2, S], F8, tag=f"x8{j}", name=f"x8{j}")
                     for j in range(NP)]
            wq8_sb = [qp.tile([128, 2, D], F8, tag=f"wq8{j}", name=f"wq8{j}")
                      for j in range(NP)]
            wk8_sb = [qp.tile([128, 2, D], F8, tag=f"wk8{j}", name=f"wk8{j}")
                      for j in range(NP)]
            for j in range(NP):
                nc.sync.dma_start(
                    out=x8_sb[j][:], in_=x8_e[j * 128:(j + 1) * 128, :])
            for j in range(NP):
                nc.sync.dma_start(
                    out=wq8_sb[j][:], in_=wq8_e[j * 128:(j + 1) * 128, :])
                nc.sync.dma_start(
                    out=wk8_sb[j][:], in_=wk8_e[j * 128:(j + 1) * 128, :])

            # ---- persistent per-core tensors ----
            Vx = [qp.tile([128, H * (HD + 1)], BF16, tag=f"vx{t}", name=f"vx{t}")
                  for t in range(NT)]
            aoT = [qp.tile([128, S], BF16, tag=f"ao{t}", name=f"ao{t}")
                   for t in range(NT)]
            # persistent exp tiles keyed by (key-tile c, head-in-pair par);
            # [0, 128c) is a static zero region, memset once here.
            exs = {}
            for c in range(NT):
                for par in range(2):
                    ex = qp.tile([128, S], BF16, tag=f"ex{c}_{par}",
                                 name=f"ex{c}_{par}")
                    exs[(c, par)] = ex
                    if c > 0:
                        nc.gpsimd.memset(ex[:, 0:128 * c], 0.0)

            # Pools shared by Q/K proj (fp8 DR), attention and epilogue.
            with tc.tile_pool(name="qk", bufs=3) as qkp, \
                 tc.tile_pool(name="stp", bufs=2) as stp, \
                 tc.tile_pool(name="rec", bufs=2) as recp:
                psc_cm = tc.tile_pool(name="pssc", bufs=2, space="PSUM")
                psc = psc_cm.__enter__()

                # WAR guard: the custom-DVE reciprocal's PSUM read of the bc
                # tile is NOT tracked by the tile framework (observed racing
                # on HW). The sc pool has bufs=2, so the second sc tile
                # allocated after bc reuses its buffer; its first matmul must
                # explicitly wait on the reciprocal.
                sc_state = {"count": 0, "pending": {}}

                def alloc_sc():
                    sc_state["count"] += 1
                    return (psc.tile([128, 1024], F32, tag="sc", name="sc"),
                            sc_state["pending"].pop(sc_state["count"], None))

                qt_tiles, kt_tiles = {}, {}

                def qk_proj_one(m, which):
                    """fp8 DoubleRow projection of Q or K output tile m."""
                    w8, dst_map, bias_col, tg = (
                        (wq8_sb, qt_tiles, bq_col, "qt") if which == 0
                        else (wk8_sb, kt_tiles, bk_col, "kt"))
                    ps, war = alloc_sc()
                    for n in range(2):
                        for j in range(NP):
                            mm = nc.tensor.matmul(
                                ps[:, n * 512:(n + 1) * 512],
                                w8[j][:, :, m * 128:(m + 1) * 128],
                                x8_sb[j][:, :, n * 512:(n + 1) * 512],
                                start=(j == 0), stop=(j == NP - 1),
                                perf_mode=DR)
                            if war is not None:
                                dep(mm, war, reason="WAR: bc recip read")
                                war = None
                    dst = qkp.tile([128, S], BF16, tag=tg, name=tg)
                    dst_map[m] = dst
                    nc.vector.tensor_scalar(
                        out=dst[:], in0=ps[:],
                        scalar1=INV_QK, scalar2=bias_col[:, m:m + 1],
                        op0=AluOp.mult, op1=AluOp.add)

                def qk_proj(m):
                    qk_proj_one(m, 0)
                    qk_proj_one(m, 1)

                # Q/K proj of the first two pairs needs only the small fp8
                # inputs - runs while the 4MB of bf16 x/Wv still streams in.
                qk_proj(0)
                qk_proj(1)

                def scores_block(t, c, par):
                    """Scores + exp + causal mask for (pair t, key tile c)."""
                    Qt, Kt = qt_tiles[t], kt_tiles[t]
                    qs = 256 * (c // 2)      # psum tile covers [qs, S)
                    q0 = 128 * c             # first valid query col
                    base = par * 64
                    sc, war = alloc_sc()
                    n0 = q0
                    while n0 < S:
                        n1 = min(qs + 512 * ((n0 - qs) // 512 + 1), S)
                        mm = nc.tensor.matmul(
                            sc[:, n0 - qs:n1 - qs],
                            Kt[base:base + 64, c * 128:(c + 1) * 128],
                            Qt[base:base + 64, n0:n1],
                            start=True, stop=True)
                        if war is not None:
                            dep(mm, war, reason="WAR: bc recip read")
                            war = None
                        n0 = n1
                    ex = exs[(c, par)]
                    nc.scalar.activation(out=ex[:, q0:S],
                                         in_=sc[:, q0 - qs:S - qs],
                                         func=Act.Exp, scale=SCALE)
                    # causal triangle on the diagonal 128-col window
                    nc.vector.tensor_mul(ex[:, q0:q0 + 128],
                                         ex[:, q0:q0 + 128], tri[:])

                # ============ Phase V: V projection (bf16) ============
                with tc.tile_pool(name="xv", bufs=1) as xp, \
                     tc.tile_pool(name="wst", bufs=8) as wp, \
                     tc.tile_pool(name="psv", bufs=4, space="PSUM") as pvp:

                    # interleave x/Wv tile loads so V-proj's c-chain can
                    # start as soon as the matching pair of tiles lands
                    xT_sb = [xp.tile([128, S], BF16, tag=f"x{c}", name=f"x{c}")
                             for c in range(NT)]
                    wv_sb = [wp.tile([128, D], BF16, tag="wv", name="wv_t")
                             for c in range(NT)]
                    for c in range(NT):
                        nc.sync.dma_start(out=xT_sb[c][:],
                                          in_=xT[c * 128:(c + 1) * 128, :])
                        nc.sync.dma_start(out=wv_sb[c][:],
                                          in_=WvT[c * 128:(c + 1) * 128, :])
                    for m in range(NT):
                        vdst = Vx[m][:].rearrange("p (h e) -> p h e", e=HD + 1)
                        nc.vector.memset(vdst[:, :, HD:HD + 1], 1.0)
                        for n in range(2):
                            ps = pvp.tile([128, 512], F32, tag="pv", name="pv")
                            for c in range(NT):
                                nc.tensor.matmul(
                                    ps[:],
                                    xT_sb[c][:, m * 128:(m + 1) * 128],
                                    wv_sb[c][:, n * 512:(n + 1) * 512],
                                    start=(c == 0), stop=False)
                            nc.tensor.matmul(ps[:], ones_row[:, :128],
                                             bv_row[:, n * 512:(n + 1) * 512],
                                             start=False, stop=True)
                            nc.vector.tensor_copy(
                                out=vdst[:, n * 8:(n + 1) * 8, 0:HD],
                                in_=ps[:].rearrange("p (h e) -> p h e", e=HD))
                        # pad mask: zero whole key rows where ids == PAD,
                        # incl. the ones column -> denominator excludes them
                        nc.vector.tensor_scalar(
                            out=Vx[m][:], in0=Vx[m][:],
                            scalar1=pad01[:, m:m + 1], scalar2=None,
                            op0=AluOp.mult)
                        # interleave pair 0's scores/exp so ACT warms up
                        # during the V projection (its AV runs in the loop)
                        scores_block(0, m, 0)
                        scores_block(0, m, 1)

                # ==== Phase A: attention, Q/K proj interleaved per pair ====
                # prefetch Wo during attention
                wo_sb = []
                for c in range(NT):
                    w_t = wop.tile([128, D], BF16, tag="wo", name="wo_t")
                    nc.sync.dma_start(out=w_t[:],
                                      in_=WoT[c * 128:(c + 1) * 128, :])
                    wo_sb.append(w_t)

                attn_stack = tc.tile_pool(name="psav", bufs=1, space="PSUM")
                pav = attn_stack.__enter__()
                for t in range(NT):        # head pair (2t, 2t+1)
                    Qt, Kt = qt_tiles[t], kt_tiles[t]
                    av_ps = {(par, g): pav.tile([HD + 1, 512], F32,
                                                tag=f"av{par}{g}",
                                                name=f"av{par}{g}")
                             for par in range(2) for g in range(2)}
                    for c in range(NT):
                        q0 = 128 * c             # first valid query col
                        for par in range(2):
                            if t > 0:
                                scores_block(t, c, par)
                            ex = exs[(c, par)]
                            for g in range(2):
                                if c <= 4 * g + 3:
                                    h = 2 * t + par
                                    # trim the chunk to the causally valid
                                    # columns (ex is zero below q0)
                                    o = max(0, q0 - 512 * g)
                                    nc.tensor.matmul(
                                        av_ps[(par, g)][:, o:512],
                                        Vx[c][:, h * (HD + 1):
                                               (h + 1) * (HD + 1)],
                                        ex[:, 512 * g + o:512 * (g + 1)],
                                        start=(c == 0),
                                        stop=(c == min(4 * g + 3, NT - 1)))

                    # epilogue: drain AV psums on ACT (reads PSUM; keeps DVE
                    # free). Denominator row (st partition 64) is broadcast
                    # to 64 partitions per head via a K=1 ones-matmul into a
                    # scores-pool psum tile, reciprocal'd on DVE, and the
                    # normalize runs on the idle GPSIMD - no DMA in the chain.
                    # epilogue: drain AV psums on ACT, broadcast the
                    # denominator row via a K=1 ones-matmul into an sc-pool
                    # psum tile, fast-reciprocal on DVE, normalize on GPSIMD
                    sts = {}
                    for par in range(2):
                        st = stp.tile([HD + 1, S], BF16, tag=f"st{par}",
                                      name=f"st{par}")
                        sts[par] = st
                        for g in range(2):
                            nc.scalar.copy(
                                out=st[:, 512 * g:512 * (g + 1)],
                                in_=av_ps[(par, g)][:])
                        nc.sync.dma_start(
                            out=aoT[t][par * 64:par * 64 + HD, :],
                            in_=st[0:HD, :])

                    if t + 2 < NT:
                        qk_proj_one(t + 2, 0)   # covers the drain latency

                    bc, war = alloc_sc()
                    for par in range(2):
                        for n in range(2):
                            mm = nc.tensor.matmul(
                                bc[par * 64:par * 64 + 64,
                                   n * 512:(n + 1) * 512],
                                ones_c[HD:HD + 1, 0:64],
                                sts[par][HD:HD + 1, n * 512:(n + 1) * 512],
                                start=True, stop=True,
                                tile_position=(64, par * 64))
                            if war is not None:
                                dep(mm, war, reason="WAR: bc recip read")
                                war = None

                    rec = recp.tile([128, S], F32, tag="rec", name="rec")
                    rcp = nc.vector.reciprocal_approx_fast(out=rec[:],
                                                           in_=bc[:])

                    if t + 2 < NT:
                        qk_proj_one(t + 2, 1)   # covers the reciprocal latency
                    sc_state["pending"][sc_state["count"] + 2] = rcp
                    if DBG and t == 0:
                        nc.sync.dma_start(out=dbg_vx[:, 0:H * (HD + 1)],
                                          in_=Vx[0][:])
                        nc.sync.dma_start(out=dbg_ex[:], in_=exs[(0, 0)][:])
                        nc.sync.dma_start(out=dbg_st[0:HD + 1, :],
                                          in_=sts[0][:])
                        nc.sync.dma_start(out=dbg_rec[:], in_=rec[:])
                    nc.gpsimd.tensor_mul(aoT[t][:], aoT[t][:], rec[:])
                    if DBG and t == 0:
                        nc.sync.dma_start(out=dbg_ao[:], in_=aoT[0][:])
                attn_stack.__exit__(None, None, None)
                psc_cm.__exit__(None, None, None)

            # ============ Phase O: output projection ============
            # (first chunks' matmuls guard the last bc reciprocal's untracked
            # psum read before its banks are recycled into the pf pool)
            with tc.tile_pool(name="ost", bufs=4) as osp, \
                 tc.tile_pool(name="psf", bufs=8, space="PSUM") as pf:
                for m in range(NT):
                    for n in range(2):
                        ps = pf.tile([128, 512], F32, tag="pf", name="psf")
                        for c in range(NT):
                            mm = nc.tensor.matmul(
                                ps[:],
                                aoT[c][:, m * 128:(m + 1) * 128],
                                wo_sb[c][:, n * 512:(n + 1) * 512],
                                start=(c == 0), stop=False)
                            if m <= 1 and c == 0 and rcp is not None:
                                dep(mm, rcp, reason="WAR: last bc recip")
                        nc.tensor.matmul(ps[:], ones_row[:, :128],
                                         bo_row[:, n * 512:(n + 1) * 512],
                                         start=False, stop=True)
                        ot = osp.tile([128, 512], F32, tag="ot", name="ot")
                        # alternate drain engines so the final chunks'
                        # copies overlap instead of queueing on one engine
                        if (2 * m + n) % 2 == 0:
                            nc.scalar.copy(out=ot[:], in_=ps[:])
                        else:
                            nc.vector.tensor_copy(out=ot[:], in_=ps[:])
                        nc.sync.dma_start(
                            out=out_e[m * 128:(m + 1) * 128,
                                      n * 512:(n + 1) * 512],
                            in_=ot[:])
    nc.finalize()
    return nc


def _host_consts():
    import ml_dtypes
    bf = ml_dtypes.bfloat16
    jj = np.arange(128)[None, :]
    pp = np.arange(128)[:, None]
    tri01 = (jj >= pp).astype(np.float32).astype(bf)   # 1 where col >= row
    return tri01


def _to_f8(a, scale):
    import ml_dtypes
    return np.asarray(
        np.clip(np.asarray(a, np.float32) * scale, -240.0, 240.0),
        dtype=ml_dtypes.float8_e4m3)


def _pair_interleave(a):
    """[D, N] -> [NP*128, 2*N]: row block j = (d rows 256j..+128 | ..+256)."""
    d, n = a.shape
    return np.ascontiguousarray(
        a.reshape(NP, 2, 128, n).transpose(0, 2, 1, 3).reshape(NP * 128, 2 * n))


def build_in_maps(x, input_ids, Wq, bq, Wk, bk, Wv, bv, Wo, bo):
    import ml_dtypes
    bf = ml_dtypes.bfloat16
    x = np.asarray(x, dtype=np.float32)
    input_ids = np.asarray(input_ids)
    tri01 = _host_consts()
    bq_r = np.ascontiguousarray(np.asarray(bq, np.float32).reshape(NT, 128).T)
    bk_r = np.ascontiguousarray(np.asarray(bk, np.float32).reshape(NT, 128).T)
    wq8 = _pair_interleave(_to_f8(np.asarray(Wq, np.float32).T, SW))
    wk8 = _pair_interleave(_to_f8(np.asarray(Wk, np.float32).T, SW))
    shared = {
        "wq8": wq8, "wk8": wk8,
        "WvT": np.ascontiguousarray(np.asarray(Wv, np.float32).T).astype(bf),
        "WoT": np.ascontiguousarray(np.asarray(Wo, np.float32).T).astype(bf),
        "bv": np.asarray(bv, np.float32).astype(bf),
        "bo": np.asarray(bo, np.float32).astype(bf),
        "ones": np.ones([S], bf),
        "tri01": tri01,
    }
    in_maps = []
    for b in range(B):
        ids_r = input_ids[b].astype(np.float32).reshape(NT, 128).T
        m = dict(shared)
        xb_T = np.ascontiguousarray(x[b].T)
        m["xT"] = xb_T.astype(bf)
        m["x8"] = _pair_interleave(_to_f8(xb_T, SX))
        m["smalls"] = np.ascontiguousarray(
            np.concatenate([ids_r, bq_r, bk_r], axis=1)).astype(np.float32)
        in_maps.append(m)
    return in_maps


def kernel(x, input_ids, Wq, bq, Wk, bk, Wv, bv, Wo, bo):
    global LAST_RESULT, LAST_EXEC_NS
    from concourse.bass_utils import run_bass_kernel_spmd

    if "nc" not in _CACHE:
        _CACHE["nc"] = _build_graph()
    nc = _CACHE["nc"]
    in_maps = build_in_maps(x, input_ids, Wq, bq, Wk, bk, Wv, bv, Wo, bo)

    trace = os.environ.get("KERNEL_TRACE", "0") == "1" and _install_trace_hook()
    res = run_bass_kernel_spmd(nc, in_maps, core_ids=list(range(B)), trace=trace)
    LAST_RESULT = res
    LAST_EXEC_NS = res.exec_time_ns
    return np.stack([res.results[b]["out"] for b in range(B)]).astype(np.float32)
